# revision 29
# baseline (speedup 1.0000x reference)
"""Trainium2 Bass kernel for LoopCoderAttention (sparse_attention).

Head-sharded tensor parallelism over 8 NeuronCores:
  core c owns query heads {2c, 2c+1} and KV head c//2.
All matmul operands are bf16 (PSUM accumulation stays fp32): enables FWL
fast weight loads, halves DMA/SBUF traffic, and keeps the PE stream dense
enough to hold the HAM clock-gate at 8/8 (2.4 GHz).

Attention is software-pipelined (depth-2 over s-tiles, both heads packed
into [128, 2, 512] PSUM tile pairs) with column-windowed diagonal/band
tiles so the PE only touches ~ the causally needed columns.

o_proj: a 2MB AllToAll reshards attention output from head-sharded to
token-sharded; the high-token half's o_proj overlaps the second AllToAll.
"""
import sys
sys.path.insert(0, '/opt/trn_rl_repo')
import numpy as np
import ml_dtypes
import concourse.bass as bass
import concourse.mybir as mybir
import concourse.tile as tile
from concourse import bacc
from concourse.bass_utils import run_bass_kernel_spmd

T = 2048
HID = 2048
HQ = 16
HK = 4
D = 128
WIN = 64
THETA = 10000.0
SCALE = D ** -0.5
NCORES = 8
TCH = 512                 # t-chunk (matmul free dim)
NCH = T // TCH            # 4 chunks
KT = HID // 128           # 16 k-tiles for 2048-deep contractions
ST = T // 128             # 16 s-tiles
TSL = T // NCORES         # 256-token output slice per core
MASKV = -1e9

F32 = mybir.dt.float32
F32R = mybir.dt.float32r
BF16 = mybir.dt.bfloat16
AF = mybir.ActivationFunctionType

_CACHE = {}


def _build():
    nc = bacc.Bacc("TRN2", target_bir_lowering=False, debug=False,
                   num_devices=NCORES)
    HST = nc.dram_tensor("HST", [HID, T], BF16, kind="ExternalInput").ap()
    WQKV = nc.dram_tensor("WQKV", [HID, 512], BF16, kind="ExternalInput").ap()
    KGT = nc.dram_tensor("KGT", [D, T], BF16, kind="ExternalInput").ap()
    VG = nc.dram_tensor("VG", [T, D], BF16, kind="ExternalInput").ap()
    WO = nc.dram_tensor("WO", [HID, HID], BF16, kind="ExternalInput").ap()
    WG = nc.dram_tensor("WG", [D, 2], BF16, kind="ExternalInput").ap()
    BG = nc.dram_tensor("BG", [1, 2], F32, kind="ExternalInput").ap()
    CSF = nc.dram_tensor("CSF", [128, T], BF16, kind="ExternalInput").ap()
    SNF = nc.dram_tensor("SNF", [128, T], BF16, kind="ExternalInput").ap()
    ONES2 = nc.dram_tensor("ONES2", [128, 4], F32, kind="ExternalInput").ap()
    IDN = nc.dram_tensor("IDN", [128, 128], BF16, kind="ExternalInput").ap()
    # head-doubled mask tables (identical halves for h0/h1)
    TRID = nc.dram_tensor("TRID", [128, 2 * 128], F32, kind="ExternalInput").ap()
    BAND0 = nc.dram_tensor("BAND0", [128, 2 * 512], F32, kind="ExternalInput").ap()
    BAND1 = nc.dram_tensor("BAND1", [128, 2 * 512], F32, kind="ExternalInput").ap()
    OUT = nc.dram_tensor("OUT", [TSL, HID], BF16, kind="ExternalOutput").ap()

    with tile.TileContext(nc) as tc:
        # pools are a strict stack: creation order is the reverse of the
        # release order at each phase boundary
        const = tc.alloc_tile_pool(name="const", bufs=1)
        dram = tc.alloc_tile_pool(name="dram", bufs=1, space="DRAM")
        aoutp = tc.alloc_tile_pool(name="aoutp", bufs=3)
        opool = tc.alloc_tile_pool(name="opool", bufs=1)
        wop = tc.alloc_tile_pool(name="wop", bufs=1)
        osb = tc.alloc_tile_pool(name="osb", bufs=2)
        work = tc.alloc_tile_pool(name="work", bufs=1)
        ropet = tc.alloc_tile_pool(name="ropet", bufs=2)
        rcpp = tc.alloc_tile_pool(name="rcpp", bufs=7)
        bcp = tc.alloc_tile_pool(name="bcp", bufs=2)
        combp = tc.alloc_tile_pool(name="combp", bufs=3)
        wqkvp = tc.alloc_tile_pool(name="wqkvp", bufs=1)
        chunkp = tc.alloc_tile_pool(name="chunkp", bufs=2)
        hsp = tc.alloc_tile_pool(name="hsp", bufs=8)
        ps1 = tc.alloc_tile_pool(name="ps1", bufs=7, space="PSUM")

        # ---- phase-1 constants first (critical path to first matmul) ----
        wqkv_sb = wqkvp.tile([128, KT, 512], BF16)
        wqkv_view = WQKV.rearrange("(k p) c -> p k c", p=128)
        hs_first = []
        n0 = NCH - 1
        for k in range(KT):
            # interleave weight/activation tiles so the k=0 matmul can
            # start after ~2 tile DMAs instead of after all 16 weight tiles
            nc.sync.dma_start(out=wqkv_sb[:, k, :], in_=wqkv_view[:, k, :])
            hs_t = hsp.tile([128, TCH], BF16, tag="hs_t", name=f"hsf{k}")
            nc.sync.dma_start(
                out=hs_t[:],
                in_=HST[k * 128:(k + 1) * 128, n0 * TCH:(n0 + 1) * TCH])
            hs_first.append(hs_t)
        csf_sb = wqkvp.tile([128, T], BF16)
        snf_sb = wqkvp.tile([128, T], BF16)
        idn_sb = wqkvp.tile([128, 128], BF16)
        wg_sb = const.tile([D, 2], BF16)
        nc.sync.dma_start(out=wg_sb[:], in_=WG)
        bg_sb = const.tile([1, 2], F32)
        nc.sync.dma_start(out=bg_sb[:], in_=BG)
        # attention-phase constants
        kgt_sb = const.tile([D, T], BF16)
        vg_sb = const.tile([128, ST, D], BF16)
        # [ones|zeros|zeros|ones]: col pair 0:2 sums h0 into psum row 0,
        # col pair 2:4 sums h1 into psum row 1 of the same bank (f32r to
        # match the f32r exsum accumulator it contracts against)
        ones_r = const.tile([128, 4], F32R)
        trid_sb = const.tile([128, 2, 128], F32)
        band0_sb = const.tile([128, 2, 512], F32)
        band1_sb = const.tile([128, 2, 512], F32)
        # o_proj weights (prefetched; consumed in phase 4)
        wo_sb = wop.tile([128, KT, HID], BF16)

        # ---- persistent work tiles (through attention) ----
        qrot = work.tile([128, 2, T], BF16)
        krot = work.tile([128, T], BF16)
        vcur = work.tile([128, ST, D], BF16)   # current v in [s, d] tiles
        gate = work.tile([8, TCH], F32)        # row 2n+h (DMA-staged access)

        a2ai_hi = dram.tile([NCORES, 2 * D, TSL // 2], BF16)
        a2ao_hi = dram.tile([NCORES, 2 * D, TSL // 2], BF16)
        a2ai_lo = dram.tile([NCORES, 2 * D, TSL // 2], BF16)
        a2ao_lo = dram.tile([NCORES, 2 * D, TSL // 2], BF16)
        a2ad_i = dram.tile([NCORES, 1, 4], BF16)
        a2ad_o = dram.tile([NCORES, 1, 4], BF16)

        def rope_chunk(dst_full, src, n):
            """dst_full[:, n*TCH:...] = neox-rope of chunk tile src [128, TCH].

            rot = src * [cos;cos] + rot90(src) * [-sin;sin], where rot90 swaps
            the two 64-partition halves (built with two SBUF->SBUF DMAs since
            DVE ops require matching base partitions).
            """
            sl = bass.ds(n * TCH, TCH)
            sr = ropet.tile([128, TCH], BF16, tag="ropesr", name=f"sr{n}")
            nc.sync.dma_start(out=sr[0:64, :], in_=src[64:128, :])
            nc.sync.dma_start(out=sr[64:128, :], in_=src[0:64, :])
            ta = ropet.tile([128, TCH], BF16, tag="ropetmp", name=f"ra{n}")
            tb = ropet.tile([128, TCH], BF16, tag="ropetmp", name=f"rb{n}")
            nc.vector.tensor_mul(ta[:], src[:], csf_sb[:, sl])
            nc.vector.tensor_mul(tb[:], sr[:], snf_sb[:, sl])
            nc.vector.tensor_add(dst_full[:, sl], ta[:], tb[:])

        # ================= phase 1: qkvT = wqkv^T @ hsT =================
        pending_small = []
        for n in reversed(range(NCH)):
            pss = [ps1.tile([128, TCH], F32, tag="ps1t", name=f"ps1_{n}_{m}")
                   for m in range(4)]
            for k in range(KT):
                if n == NCH - 1:
                    hs_t = hs_first[k]
                else:
                    hs_t = hsp.tile([128, TCH], BF16, tag="hs_t",
                                    name=f"hs_{n}_{k}")
                    nc.sync.dma_start(
                        out=hs_t[:],
                        in_=HST[k * 128:(k + 1) * 128,
                                n * TCH:(n + 1) * TCH])
                for m in range(4):
                    nc.tensor.matmul(pss[m][:],
                                     wqkv_sb[:, k, m * 128:(m + 1) * 128],
                                     hs_t[:],
                                     start=(k == 0), stop=(k == KT - 1))
            if n == NCH - 1:
                # rope tables + identity: after the hot first-chunk DMAs,
                # before their first readers below
                nc.sync.dma_start(out=csf_sb[:], in_=CSF)
                nc.sync.dma_start(out=snf_sb[:], in_=SNF)
                nc.sync.dma_start(out=idn_sb[:], in_=IDN)
            if n == 1:
                # attention constants, needed right at attention start
                nc.sync.dma_start(out=kgt_sb[:], in_=KGT)
                nc.sync.dma_start(
                    out=vg_sb[:],
                    in_=VG.rearrange("(s p) d -> p s d", p=128))
                nc.sync.dma_start(out=ones_r[:], in_=ONES2.bitcast(F32R))
            if n == 0:
                nc.sync.dma_start(out=trid_sb[:],
                                  in_=TRID.rearrange("p (h c) -> p h c", h=2))
                nc.sync.dma_start(out=band0_sb[:],
                                  in_=BAND0.rearrange("p (h c) -> p h c", h=2))
                nc.sync.dma_start(out=band1_sb[:],
                                  in_=BAND1.rearrange("p (h c) -> p h c", h=2))
            if pending_small:
                pending_small.pop(0)()
            sl = bass.ds(n * TCH, TCH)
            q0c = chunkp.tile([128, TCH], BF16, tag="q0c")
            q1c = chunkp.tile([128, TCH], BF16, tag="q1c")
            kc = chunkp.tile([128, TCH], BF16, tag="kc")
            vc = chunkp.tile([128, TCH], BF16, tag="vc")
            nc.scalar.activation(q0c[:], pss[0][:], AF.Copy)
            nc.scalar.activation(q1c[:], pss[1][:], AF.Copy)
            nc.scalar.activation(kc[:], pss[2][:], AF.Copy)
            nc.vector.tensor_copy(vc[:], pss[3][:])

            rope_chunk(qrot[:, 0, :], q0c, n)
            rope_chunk(qrot[:, 1, :], q1c, n)
            rope_chunk(krot, kc, n)

            def small_ops(n=n, vc=vc, sl=sl):
                # v transposes + gates for chunk n: emitted one chunk later so
                # the PE stream never waits on the DVE rope/copy latency
                for j in range(4):
                    s = 4 * n + j
                    pt = ps1.tile([128, 128], BF16, tag="ps1g",
                                  name=f"pt{s}", bufs=1)
                    nc.tensor.transpose(pt[:], vc[:, j * 128:(j + 1) * 128],
                                        idn_sb[:])
                    nc.vector.tensor_copy(vcur[:, s, :], pt[:])
                for h in range(2):
                    r = 2 * n + h
                    gp = ps1.tile([1, TCH], F32, tag="ps1g",
                                  name=f"gp{r}", bufs=1)
                    nc.tensor.matmul(gp[:], wg_sb[:, h:h + 1], qrot[:, h, sl],
                                     start=True, stop=True)
                    gst = chunkp.tile([1, TCH], F32, tag="gst", name=f"gst{r}")
                    nc.scalar.activation(gst[:], gp[:], AF.Sigmoid,
                                         bias=bg_sb[0:1, h:h + 1])
                    nc.sync.dma_start(out=gate[r:r + 1, :], in_=gst[:])

            pending_small.append(small_ops)

        for f in pending_small:
            f()
        pending_small.clear()

        # prefetch o_proj weights now (after all phase-1 input DMAs queued)
        for k in range(KT):
            nc.sync.dma_start(out=wo_sb[:, k, :],
                              in_=WO[k * 128:(k + 1) * 128, :])

        ps1.release()
        hsp.release()
        chunkp.release()
        wqkvp.release()

        afull_hi = opool.tile([128, KT, TSL // 2], BF16)
        afull_lo = opool.tile([128, KT, TSL // 2], BF16)

        drainp = tc.alloc_tile_pool(name="drainp", bufs=3)
        exsp = tc.alloc_tile_pool(name="exsp", bufs=2)
        expp = tc.alloc_tile_pool(name="expp", bufs=4)
        psqk = tc.alloc_tile_pool(name="psqk", bufs=2, space="PSUM")
        pspv = tc.alloc_tile_pool(name="pspv", bufs=1, space="PSUM")
        pssm = tc.alloc_tile_pool(name="pssm", bufs=1, space="PSUM")

        # ============ phase 2: attention (global + local) ============
        # warm the collective path with a tiny all-to-all so the first real
        # one doesn't pay the ~11us cross-core trigger handshake
        nc.gpsimd.collective_compute(
            "AllToAll", mybir.AluOpType.bypass,
            replica_groups=[list(range(NCORES))],
            ins=[a2ad_i[:].opt()], outs=[a2ad_o[:].opt()])

        # chunks descend so the high-token half finishes first and its
        # all-to-all overlaps the low-token half's compute. All 8 passes are
        # software-pipelined into ONE flat PE stream (depth 2 across pass
        # boundaries) so the PE never idles long enough to re-throttle HAM.
        class Pass:
            def __init__(self, n, lhs_of, v_of, tiles, pfx):
                self.n, self.lhs_of, self.v_of = n, lhs_of, v_of
                self.tiles, self.pfx = tiles, pfx
                self.n_t = len(tiles)
                self.exs = {}
                self.pv = None
                self.exsum = None
                self.pvsb = None
                self.smsb = None

            def emit_qk(self, i):
                n, pfx = self.n, self.pfx
                s, w, W, m_ap, m_w = self.tiles[i]
                qk = psqk.tile([128, 2, TCH], F32, tag="qk",
                               name=f"qk{pfx}_{s}")
                for h in range(2):
                    nc.tensor.matmul(
                        qk[:, h, w:w + W], self.lhs_of(s),
                        qrot[:, h, bass.ds(n * TCH + w, W)],
                        start=True, stop=True)
                if m_ap is not None:
                    nc.vector.tensor_add(qk[:, :, w:w + m_w],
                                         qk[:, :, w:w + m_w], m_ap)
                ex = expp.tile([128, 2, TCH], BF16, tag="ex",
                               name=f"ex{pfx}_{s}")
                nc.scalar.activation(ex[:, :, w:w + W], qk[:, :, w:w + W],
                                     AF.Exp, scale=SCALE)
                self.exs[i] = ex

            def emit_pv(self, i):
                s, w, W, m_ap, m_w = self.tiles[i]
                ex = self.exs.pop(i)
                first = (i == 0)
                last = (i == self.n_t - 1)
                if first:
                    self.pv = pspv.tile([128, 2, TCH], F32, tag="pv",
                                        name=f"pv{self.pfx}")
                    self.exsum = exsp.tile([128, 2, TCH], F32R, tag="exsum",
                                           name=f"exs{self.pfx}")
                for h in range(2):
                    nc.tensor.matmul(self.pv[:, h, w:w + W], self.v_of(s),
                                     ex[:, h, w:w + W],
                                     start=first, stop=last)
                # running sum of exp tiles (DVE) -> one tiny PE colsum at
                # finish; halves the PE row count of the softmax epilogue
                if first:
                    nc.vector.tensor_copy(self.exsum[:], ex[:])
                else:
                    nc.vector.tensor_add(self.exsum[:, :, w:w + W],
                                         self.exsum[:, :, w:w + W],
                                         ex[:, :, w:w + W])
                if last:
                    self.finish()

            def finish(self):
                pfx = self.pfx
                # denominators: colsum of exsum (h0 -> row 0, h1 -> row 1)
                smt = pssm.tile([2, TCH], F32, tag="sm", name=f"sm{pfx}")
                for h in range(2):
                    nc.tensor.matmul(smt[0:2, :], ones_r[:, 2 * h:2 * h + 2],
                                     self.exsum[:, h, :],
                                     start=(h == 0), stop=(h == 1))
                self.smsb = rcpp.tile([2, TCH], F32, tag="rcp2",
                                      name=f"smsb{pfx}")
                nc.scalar.activation(self.smsb[:], smt[:], AF.Copy)
                # drain pv to sbuf bf16 (frees the psum pair for next pass)
                self.pvsb = drainp.tile([128, 2, TCH], BF16, tag="pvsb",
                                        name=f"pvsb{pfx}")
                nc.scalar.activation(self.pvsb[:], self.pv[:], AF.Copy)

        cstate = {}

        def combine_g(n, gp):
            # gate-weighted normalized global half: t1 = pv_g * gate/sum_g.
            # Runs right after the global pass finishes, hidden behind the
            # local pass, so the post-chunk serial tail is only combine_l.
            gsl_t = rcpp.tile([2, TCH], F32, tag="rcp2", name=f"gsl{n}")
            nc.sync.dma_start(out=gsl_t[:], in_=gate[2 * n:2 * n + 2, :])
            g1 = rcpp.tile([2, TCH], F32, tag="rcp2", name=f"g1{n}")
            nc.vector.tensor_scalar(g1[:], gsl_t[:], -1.0, 1.0,
                                    mybir.AluOpType.mult,
                                    mybir.AluOpType.add)
            rg = rcpp.tile([2, TCH], F32, tag="rcp2", name=f"rg{n}")
            nc.vector.reciprocal_approx_fast(rg[:], gp.smsb[:])
            ag = rcpp.tile([2, TCH], BF16, tag="rcpb", name=f"ag{n}")
            nc.vector.tensor_mul(ag[:], gsl_t[:], rg[:])
            # head-1 row to base partition 0 for partition_broadcast
            ag1 = rcpp.tile([1, TCH], BF16, tag="rcpb1", name=f"ag1{n}")
            nc.sync.dma_start(out=ag1[:], in_=ag[1:2, :])
            bg_t = bcp.tile([128, 2, TCH], BF16, tag="bcast", name=f"bg{n}")
            nc.gpsimd.partition_broadcast(bg_t[:, 0, :], ag[0:1, :])
            nc.gpsimd.partition_broadcast(bg_t[:, 1, :], ag1[:])
            t1 = combp.tile([128, 2, TCH], BF16, tag="comb", name=f"t1{n}")
            nc.vector.tensor_mul(t1[:], gp.pvsb[:], bg_t[:])
            cstate[n] = (g1, t1)

        def combine_l(n, lp):
            g1, t1 = cstate.pop(n)
            rl = rcpp.tile([2, TCH], F32, tag="rcp2", name=f"rl{n}")
            nc.vector.reciprocal_approx_fast(rl[:], lp.smsb[:])
            al = rcpp.tile([2, TCH], BF16, tag="rcpb", name=f"al{n}")
            nc.vector.tensor_mul(al[:], g1[:], rl[:])
            al1 = rcpp.tile([1, TCH], BF16, tag="rcpb1", name=f"al1{n}")
            nc.sync.dma_start(out=al1[:], in_=al[1:2, :])
            bl_t = bcp.tile([128, 2, TCH], BF16, tag="bcast", name=f"bl{n}")
            nc.gpsimd.partition_broadcast(bl_t[:, 0, :], al[0:1, :])
            nc.gpsimd.partition_broadcast(bl_t[:, 1, :], al1[:])
            t2 = combp.tile([128, 2, TCH], BF16, tag="comb", name=f"t2{n}")
            ao = aoutp.tile([128, 2, TCH], BF16, tag="aout", name=f"ao{n}")
            nc.vector.tensor_mul(t2[:], lp.pvsb[:], bl_t[:])
            nc.vector.tensor_add(ao[:], t1[:], t2[:])

            # ship finished 128-col blocks to a2a staging
            # token 1024+128c (hi) / 128c (lo) lives in chunk n at column
            # offset 128j; each unit covers 4 destination quarter-blocks
            buf = a2ai_hi if n >= 2 else a2ai_lo
            c0 = (n - 2) * 4 if n >= 2 else n * 4
            for h in range(2):
                for j in range(4):
                    nc.sync.dma_start(
                        out=buf[c0 + j, h * D:(h + 1) * D, :],
                        in_=ao[:, h, j * 128:(j + 1) * 128])

            if n == 2:
                # all-to-all #1: high-token halves (overlaps chunks 1,0)
                nc.gpsimd.collective_compute(
                    "AllToAll", mybir.AluOpType.bypass,
                    replica_groups=[list(range(NCORES))],
                    ins=[a2ai_hi[:].opt()], outs=[a2ao_hi[:].opt()])
                # stage its result immediately so o_proj-hi starts with no
                # gap the moment the attention stream drains
                nc.sync.dma_start(
                    out=afull_hi[:],
                    in_=a2ao_hi[:].rearrange("c p n -> (c p) n")
                        .rearrange("(k p) n -> p k n", p=128))

        units = []   # flat (pass, tile_idx) stream; post-pass hooks fire
        hooks = {}   # after the pv of a pass's last tile is emitted
        for n in reversed(range(NCH)):
            gtiles = []
            for s in range(0, 4 * n + 4):
                j = s - 4 * n
                if j < 0:
                    gtiles.append((s, 0, TCH, None, 0))
                else:
                    w = 128 * j
                    gtiles.append((s, w, TCH - w, trid_sb[:], 128))
            gp = Pass(n, lambda s: kgt_sb[:, s * 128:(s + 1) * 128],
                      lambda s: vg_sb[:, s, :], gtiles, f"g{n}")
            ltiles = []
            rlo = -1 if n > 0 else 0
            for r in range(rlo, 4):
                s = 4 * n + r
                if r == rlo:
                    m_ap = band1_sb[:] if r == -1 else band0_sb[:]
                    ltiles.append((s, 0, TCH, m_ap, TCH))
                elif r < 3:
                    w = 128 * r
                    ltiles.append((s, w, 256, band0_sb[:, :, 0:256], 256))
                else:
                    ltiles.append((s, 384, 128, band0_sb[:, :, 0:128], 128))
            lp = Pass(n, lambda s: krot[:, s * 128:(s + 1) * 128],
                      lambda s: vcur[:, s, :], ltiles, f"l{n}")
            units.extend((gp, i) for i in range(gp.n_t))
            hooks[len(units) - 1] = (combine_g, n, gp)
            units.extend((lp, i) for i in range(lp.n_t))
            hooks[len(units) - 1] = (combine_l, n, lp)

        DEPTH = 2
        for u, (p, i) in enumerate(units):
            p.emit_qk(i)
            if u >= DEPTH:
                pp, pi = units[u - DEPTH]
                pp.emit_pv(pi)
                if (u - DEPTH) in hooks:
                    fn, hn, hp = hooks[u - DEPTH]
                    fn(hn, hp)
        for u in range(max(0, len(units) - DEPTH), len(units)):
            pp, pi = units[u]
            pp.emit_pv(pi)
            if u in hooks:
                fn, hn, hp = hooks[u]
                fn(hn, hp)

        pssm.release()
        pspv.release()
        psqk.release()
        expp.release()
        exsp.release()
        drainp.release()
        combp.release()
        bcp.release()
        rcpp.release()
        ropet.release()
        work.release()

        # ===== phase 3: all-to-all #2 (low halves) + split o_proj =====
        nc.gpsimd.collective_compute(
            "AllToAll", mybir.AluOpType.bypass,
            replica_groups=[list(range(NCORES))],
            ins=[a2ai_lo[:].opt()], outs=[a2ao_lo[:].opt()])

        pso = tc.alloc_tile_pool(name="pso", bufs=2, space="PSUM")

        nc.sync.dma_start(
            out=afull_lo[:],
            in_=a2ao_lo[:].rearrange("c p n -> (c p) n")
                .rearrange("(k p) n -> p k n", p=128))

        # ============ phase 4: o_proj for our token slice ============
        # hi half first: its matmuls run while all-to-all #2 is in flight.
        # OUT rows 0-127 = low half-slice, rows 128-255 = high half-slice
        for tt, afull in ((1, afull_hi), (0, afull_lo)):
            po = pso.tile([128, 4, TCH], F32, tag="po", name=f"po{tt}")
            for k in range(KT):
                for e in range(NCH):
                    nc.tensor.matmul(po[:, e, :],
                                     afull[:, k, :],
                                     wo_sb[:, k, e * TCH:(e + 1) * TCH],
                                     start=(k == 0), stop=(k == KT - 1))
            # per-e drains so each column block ships while the PE finishes
            # the remaining stop-matmuls / the other half's contraction
            for e in range(NCH):
                ot = osb.tile([128, TCH], BF16, tag="ot", name=f"ot{tt}_{e}")
                nc.scalar.activation(ot[:], po[:, e, :], AF.Copy)
                nc.sync.dma_start(
                    out=OUT[tt * 128:(tt + 1) * 128,
                            e * TCH:(e + 1) * TCH], in_=ot[:])
        pso.release()
        osb.release()
        wop.release()
        opool.release()
        aoutp.release()
        dram.release()
        const.release()

    nc.compile()
    return nc


def _host_prep(hidden_states, positions, k_global, v_global, w_qkv, w_o,
               w_gate, b_gate):
    """Layout-only host transforms + constant tables -> per-core in_maps."""
    f32 = np.float32
    bf = ml_dtypes.bfloat16
    hs = np.asarray(hidden_states, f32)
    pos = np.asarray(positions)
    kg = np.asarray(k_global, f32)
    vg = np.asarray(v_global, f32)
    wqkv = np.asarray(w_qkv, f32)
    wo = np.ascontiguousarray(np.asarray(w_o, f32).astype(bf))
    wg = np.asarray(w_gate, f32)
    bg = np.asarray(b_gate, f32)

    hst = np.ascontiguousarray(hs.T.astype(bf))

    half = D // 2
    inv_freq = (THETA ** (-np.arange(half, dtype=f32) / half)).astype(f32)
    ang = pos.astype(f32)[:, None] * inv_freq[None, :]
    cos_t = np.cos(ang).astype(f32).T       # [64, T]
    sin_t = np.sin(ang).astype(f32).T
    csf = np.ascontiguousarray(np.concatenate([cos_t, cos_t], axis=0).astype(bf))
    snf = np.ascontiguousarray(np.concatenate([-sin_t, sin_t], axis=0).astype(bf))

    p = np.arange(128, dtype=np.int64)[:, None]
    c128 = np.arange(128, dtype=np.int64)[None, :]
    c512 = np.arange(512, dtype=np.int64)[None, :]
    # TRI[p, c] = 0 iff c >= p  (diagonal 128-block of the causal mask)
    tri = np.where(c128 - p >= 0, 0.0, MASKV).astype(f32)
    # BAND0[p, c] = 0 iff 0 <= c - p <= WIN   (local band, d0 = 0)
    band0 = np.where((c512 - p >= 0) & (c512 - p <= WIN), 0.0, MASKV).astype(f32)
    # BAND1[p, c] = 0 iff 0 <= 128 + c - p <= WIN  (local band, d0 = 128)
    band1 = np.where((128 + c512 - p >= 0) & (128 + c512 - p <= WIN),
                     0.0, MASKV).astype(f32)
    trid = np.ascontiguousarray(np.concatenate([tri, tri], axis=1))
    band0d = np.ascontiguousarray(np.concatenate([band0, band0], axis=1))
    band1d = np.ascontiguousarray(np.concatenate([band1, band1], axis=1))

    on = np.ones((128, 1), f32)
    zo = np.zeros((128, 1), f32)
    ones2 = np.ascontiguousarray(np.concatenate([on, zo, zo, on], axis=1))
    idn = np.eye(128, dtype=f32).astype(bf)

    in_maps = []
    for c in range(NCORES):
        g = c // 2
        wq = wqkv[:, 2 * c * D:(2 * c + 2) * D]
        wk = wqkv[:, HQ * D + g * D:HQ * D + (g + 1) * D]
        wv = wqkv[:, (HQ + HK) * D + g * D:(HQ + HK) * D + (g + 1) * D]
        in_maps.append({
            "HST": hst,
            "WQKV": np.ascontiguousarray(
                np.concatenate([wq, wk, wv], axis=1).astype(bf)),
            "KGT": np.ascontiguousarray(kg[:, g * D:(g + 1) * D].T.astype(bf)),
            "VG": np.ascontiguousarray(vg[:, g * D:(g + 1) * D].astype(bf)),
            "WO": wo,
            "WG": np.ascontiguousarray(wg[:, 2 * c:2 * c + 2].astype(bf)),
            "BG": np.ascontiguousarray(bg[2 * c:2 * c + 2].reshape(1, 2)),
            "CSF": csf,
            "SNF": snf,
            "ONES2": ones2,
            "IDN": idn,
            "TRID": trid,
            "BAND0": band0d,
            "BAND1": band1d,
        })
    return in_maps


def kernel(**inputs):
    if "nc" not in _CACHE:
        _CACHE["nc"] = _build()
    nc = _CACHE["nc"]
    in_maps = _host_prep(**inputs)
    res = run_bass_kernel_spmd(nc, in_maps, core_ids=list(range(NCORES)))
    out = np.empty((T, HID), np.float32)
    for c in range(NCORES):
        o = np.asarray(res.results[c]["OUT"]).astype(np.float32)
        out[128 * c:128 * (c + 1)] = o[0:128]
        out[1024 + 128 * c:1024 + 128 * (c + 1)] = o[128:256]
    return out


# revision 32
# speedup vs baseline: 1.0460x; 1.0460x over previous
"""Trainium2 Bass kernel for LoopCoderAttention (sparse_attention).

Head-sharded tensor parallelism over 8 NeuronCores:
  core c owns query heads {2c, 2c+1} and KV head c//2.
All matmul operands are bf16 (PSUM accumulation stays fp32): enables FWL
fast weight loads, halves DMA/SBUF traffic, and keeps the PE stream dense
enough to hold the HAM clock-gate at 8/8 (2.4 GHz).

Attention is software-pipelined (depth-2 over s-tiles, both heads packed
into [128, 2, 512] PSUM tile pairs) with column-windowed diagonal/band
tiles so the PE only touches ~ the causally needed columns.

o_proj: a 2MB AllToAll reshards attention output from head-sharded to
token-sharded; the high-token half's o_proj overlaps the second AllToAll.
"""
import sys
sys.path.insert(0, '/opt/trn_rl_repo')
import numpy as np
import ml_dtypes
import concourse.bass as bass
import concourse.mybir as mybir
import concourse.tile as tile
from concourse import bacc
from concourse.bass_utils import run_bass_kernel_spmd

T = 2048
HID = 2048
HQ = 16
HK = 4
D = 128
WIN = 64
THETA = 10000.0
SCALE = D ** -0.5
NCORES = 8
TCH = 512                 # t-chunk (matmul free dim)
NCH = T // TCH            # 4 chunks
KT = HID // 128           # 16 k-tiles for 2048-deep contractions
ST = T // 128             # 16 s-tiles
TSL = T // NCORES         # 256-token output slice per core
MASKV = -1e9

F32 = mybir.dt.float32
F32R = mybir.dt.float32r
BF16 = mybir.dt.bfloat16
AF = mybir.ActivationFunctionType

_CACHE = {}


def _build():
    nc = bacc.Bacc("TRN2", target_bir_lowering=False, debug=False,
                   num_devices=NCORES)
    HST = nc.dram_tensor("HST", [HID, T], BF16, kind="ExternalInput").ap()
    WQKV = nc.dram_tensor("WQKV", [HID, 512], BF16, kind="ExternalInput").ap()
    KGT = nc.dram_tensor("KGT", [D, T], BF16, kind="ExternalInput").ap()
    VG = nc.dram_tensor("VG", [T, D], BF16, kind="ExternalInput").ap()
    WO = nc.dram_tensor("WO", [HID, HID], BF16, kind="ExternalInput").ap()
    WG = nc.dram_tensor("WG", [D, 2], BF16, kind="ExternalInput").ap()
    BG = nc.dram_tensor("BG", [1, 2], F32, kind="ExternalInput").ap()
    CSF = nc.dram_tensor("CSF", [128, T], BF16, kind="ExternalInput").ap()
    SNF = nc.dram_tensor("SNF", [128, T], BF16, kind="ExternalInput").ap()
    ONES2 = nc.dram_tensor("ONES2", [128, 4], F32, kind="ExternalInput").ap()
    IDN = nc.dram_tensor("IDN", [128, 128], BF16, kind="ExternalInput").ap()
    # head-doubled mask tables (identical halves for h0/h1)
    TRID = nc.dram_tensor("TRID", [128, 2 * 128], F32, kind="ExternalInput").ap()
    BAND0 = nc.dram_tensor("BAND0", [128, 2 * 512], F32, kind="ExternalInput").ap()
    BAND1 = nc.dram_tensor("BAND1", [128, 2 * 512], F32, kind="ExternalInput").ap()
    OUT = nc.dram_tensor("OUT", [TSL, HID], BF16, kind="ExternalOutput").ap()

    with tile.TileContext(nc) as tc:
        # pools are a strict stack: creation order is the reverse of the
        # release order at each phase boundary
        const = tc.alloc_tile_pool(name="const", bufs=1)
        dram = tc.alloc_tile_pool(name="dram", bufs=1, space="DRAM")
        aoutp = tc.alloc_tile_pool(name="aoutp", bufs=3)
        opool = tc.alloc_tile_pool(name="opool", bufs=1)
        wop = tc.alloc_tile_pool(name="wop", bufs=1)
        osb = tc.alloc_tile_pool(name="osb", bufs=2)
        work = tc.alloc_tile_pool(name="work", bufs=1)
        ropet = tc.alloc_tile_pool(name="ropet", bufs=2)
        rcpp = tc.alloc_tile_pool(name="rcpp", bufs=7)
        bcp = tc.alloc_tile_pool(name="bcp", bufs=2)
        combp = tc.alloc_tile_pool(name="combp", bufs=3)
        wqkvp = tc.alloc_tile_pool(name="wqkvp", bufs=1)
        chunkp = tc.alloc_tile_pool(name="chunkp", bufs=2)
        hsp = tc.alloc_tile_pool(name="hsp", bufs=8)
        ps1 = tc.alloc_tile_pool(name="ps1", bufs=7, space="PSUM")

        # ---- phase-1 constants first (critical path to first matmul) ----
        wqkv_sb = wqkvp.tile([128, KT, 512], BF16)
        wqkv_view = WQKV.rearrange("(k p) c -> p k c", p=128)
        hs_first = []
        n0 = NCH - 1
        for k in range(KT):
            # interleave weight/activation tiles so the k=0 matmul can
            # start after ~2 tile DMAs instead of after all 16 weight tiles
            nc.sync.dma_start(out=wqkv_sb[:, k, :], in_=wqkv_view[:, k, :])
            hs_t = hsp.tile([128, TCH], BF16, tag="hs_t", name=f"hsf{k}")
            nc.sync.dma_start(
                out=hs_t[:],
                in_=HST[k * 128:(k + 1) * 128, n0 * TCH:(n0 + 1) * TCH])
            hs_first.append(hs_t)
        csf_sb = wqkvp.tile([128, T], BF16)
        snf_sb = wqkvp.tile([128, T], BF16)
        idn_sb = wqkvp.tile([128, 128], BF16)
        wg_sb = const.tile([D, 2], BF16)
        nc.sync.dma_start(out=wg_sb[:], in_=WG)
        bg_sb = const.tile([1, 2], F32)
        nc.sync.dma_start(out=bg_sb[:], in_=BG)
        # attention-phase constants
        kgt_sb = const.tile([D, T], BF16)
        vg_sb = const.tile([128, ST, D], BF16)
        # [ones|zeros|zeros|ones]: col pair 0:2 sums h0 into psum row 0,
        # col pair 2:4 sums h1 into psum row 1 of the same bank (f32r to
        # match the f32r exsum accumulator it contracts against)
        ones_r = const.tile([128, 4], F32R)
        trid_sb = const.tile([128, 2, 128], F32)
        band0_sb = const.tile([128, 2, 512], F32)
        band1_sb = const.tile([128, 2, 512], F32)
        # o_proj weights (prefetched; consumed in phase 4)
        wo_sb = wop.tile([128, KT, HID], BF16)

        # ---- persistent work tiles (through attention) ----
        qrot = work.tile([128, 2, T], BF16)
        krot = work.tile([128, T], BF16)
        vcur = work.tile([128, ST, D], BF16)   # current v in [s, d] tiles
        gate = work.tile([8, TCH], F32)        # row 2n+h (DMA-staged access)

        a2ai_hi = dram.tile([NCORES, 2 * D, TSL // 2], BF16)
        a2ao_hi = dram.tile([NCORES, 2 * D, TSL // 2], BF16)
        a2ai_lo = dram.tile([NCORES, 2 * D, TSL // 2], BF16)
        a2ao_lo = dram.tile([NCORES, 2 * D, TSL // 2], BF16)
        a2ad_i = dram.tile([NCORES, 1, 4], BF16)
        a2ad_o = dram.tile([NCORES, 1, 4], BF16)

        def rope_chunk(dst_full, src, n):
            """dst_full[:, n*TCH:...] = neox-rope of chunk tile src [128, TCH].

            rot = src * [cos;cos] + rot90(src) * [-sin;sin], where rot90 swaps
            the two 64-partition halves (built with two SBUF->SBUF DMAs since
            DVE ops require matching base partitions).
            """
            sl = bass.ds(n * TCH, TCH)
            sr = ropet.tile([128, TCH], BF16, tag="ropesr", name=f"sr{n}")
            nc.sync.dma_start(out=sr[0:64, :], in_=src[64:128, :])
            nc.sync.dma_start(out=sr[64:128, :], in_=src[0:64, :])
            ta = ropet.tile([128, TCH], BF16, tag="ropetmp", name=f"ra{n}")
            tb = ropet.tile([128, TCH], BF16, tag="ropetmp", name=f"rb{n}")
            nc.vector.tensor_mul(ta[:], src[:], csf_sb[:, sl])
            nc.vector.tensor_mul(tb[:], sr[:], snf_sb[:, sl])
            nc.vector.tensor_add(dst_full[:, sl], ta[:], tb[:])

        # ================= phase 1: qkvT = wqkv^T @ hsT =================
        pending_small = []
        for n in reversed(range(NCH)):
            pss = [ps1.tile([128, TCH], F32, tag="ps1t", name=f"ps1_{n}_{m}")
                   for m in range(4)]
            hs_n = []
            for k in range(KT):
                if n == NCH - 1:
                    hs_t = hs_first[k]
                else:
                    hs_t = hsp.tile([128, TCH], BF16, tag="hs_t",
                                    name=f"hs_{n}_{k}")
                    nc.sync.dma_start(
                        out=hs_t[:],
                        in_=HST[k * 128:(k + 1) * 128,
                                n * TCH:(n + 1) * TCH])
                hs_n.append(hs_t)
            # m=3's psum bank is the previous chunk's most recently freed
            # slot; emit its first k-tiles after m=0..2's so the in-order PE
            # never blocks on the bank hand-off at the chunk boundary
            DEFER = 4 if n < NCH - 1 else 0
            order = [(k, m) for k in range(DEFER) for m in range(3)]
            order += [(k, 3) for k in range(DEFER)]
            order += [(k, m) for k in range(DEFER, KT) for m in range(4)]
            for k, m in order:
                nc.tensor.matmul(pss[m][:],
                                 wqkv_sb[:, k, m * 128:(m + 1) * 128],
                                 hs_n[k][:],
                                 start=(k == 0), stop=(k == KT - 1))
            if n == NCH - 1:
                # rope tables + identity: after the hot first-chunk DMAs,
                # before their first readers below
                nc.sync.dma_start(out=csf_sb[:], in_=CSF)
                nc.sync.dma_start(out=snf_sb[:], in_=SNF)
                nc.sync.dma_start(out=idn_sb[:], in_=IDN)
            if n == 1:
                # attention constants, needed right at attention start
                nc.sync.dma_start(out=kgt_sb[:], in_=KGT)
                nc.sync.dma_start(
                    out=vg_sb[:],
                    in_=VG.rearrange("(s p) d -> p s d", p=128))
                nc.sync.dma_start(out=ones_r[:], in_=ONES2.bitcast(F32R))
            if n == 0:
                nc.sync.dma_start(out=trid_sb[:],
                                  in_=TRID.rearrange("p (h c) -> p h c", h=2))
                nc.sync.dma_start(out=band0_sb[:],
                                  in_=BAND0.rearrange("p (h c) -> p h c", h=2))
                nc.sync.dma_start(out=band1_sb[:],
                                  in_=BAND1.rearrange("p (h c) -> p h c", h=2))
            if pending_small:
                pending_small.pop(0)()
            sl = bass.ds(n * TCH, TCH)
            q0c = chunkp.tile([128, TCH], BF16, tag="q0c")
            q1c = chunkp.tile([128, TCH], BF16, tag="q1c")
            kc = chunkp.tile([128, TCH], BF16, tag="kc")
            vc = chunkp.tile([128, TCH], BF16, tag="vc")
            nc.scalar.activation(q0c[:], pss[0][:], AF.Copy)
            nc.scalar.activation(q1c[:], pss[1][:], AF.Copy)
            nc.scalar.activation(kc[:], pss[2][:], AF.Copy)
            nc.vector.tensor_copy(vc[:], pss[3][:])

            rope_chunk(qrot[:, 0, :], q0c, n)
            rope_chunk(qrot[:, 1, :], q1c, n)
            rope_chunk(krot, kc, n)

            def small_ops(n=n, vc=vc, sl=sl):
                # v transposes + gates for chunk n: emitted one chunk later so
                # the PE stream never waits on the DVE rope/copy latency
                for j in range(4):
                    s = 4 * n + j
                    pt = ps1.tile([128, 128], BF16, tag="ps1g",
                                  name=f"pt{s}", bufs=1)
                    nc.tensor.transpose(pt[:], vc[:, j * 128:(j + 1) * 128],
                                        idn_sb[:])
                    nc.vector.tensor_copy(vcur[:, s, :], pt[:])
                for h in range(2):
                    r = 2 * n + h
                    gp = ps1.tile([1, TCH], F32, tag="ps1g",
                                  name=f"gp{r}", bufs=1)
                    nc.tensor.matmul(gp[:], wg_sb[:, h:h + 1], qrot[:, h, sl],
                                     start=True, stop=True)
                    gst = chunkp.tile([1, TCH], F32, tag="gst", name=f"gst{r}")
                    nc.scalar.activation(gst[:], gp[:], AF.Sigmoid,
                                         bias=bg_sb[0:1, h:h + 1])
                    nc.sync.dma_start(out=gate[r:r + 1, :], in_=gst[:])

            pending_small.append(small_ops)

        for f in pending_small:
            f()
        pending_small.clear()

        # prefetch o_proj weights now (after all phase-1 input DMAs queued)
        for k in range(KT):
            nc.sync.dma_start(out=wo_sb[:, k, :],
                              in_=WO[k * 128:(k + 1) * 128, :])

        ps1.release()
        hsp.release()
        chunkp.release()
        wqkvp.release()

        afull_hi = opool.tile([128, KT, TSL // 2], BF16)
        afull_lo = opool.tile([128, KT, TSL // 2], BF16)

        drainp = tc.alloc_tile_pool(name="drainp", bufs=3)
        exsp = tc.alloc_tile_pool(name="exsp", bufs=2)
        expp = tc.alloc_tile_pool(name="expp", bufs=4)
        psqk = tc.alloc_tile_pool(name="psqk", bufs=2, space="PSUM")
        pspv = tc.alloc_tile_pool(name="pspv", bufs=1, space="PSUM")
        pssm = tc.alloc_tile_pool(name="pssm", bufs=1, space="PSUM")

        # ============ phase 2: attention (global + local) ============
        # warm the collective path with a tiny all-to-all so the first real
        # one doesn't pay the ~11us cross-core trigger handshake
        nc.gpsimd.collective_compute(
            "AllToAll", mybir.AluOpType.bypass,
            replica_groups=[list(range(NCORES))],
            ins=[a2ad_i[:].opt()], outs=[a2ad_o[:].opt()])

        # chunks descend so the high-token half finishes first and its
        # all-to-all overlaps the low-token half's compute. All 8 passes are
        # software-pipelined into ONE flat PE stream (depth 2 across pass
        # boundaries) so the PE never idles long enough to re-throttle HAM.
        class Pass:
            def __init__(self, n, lhs_of, v_of, tiles, pfx):
                self.n, self.lhs_of, self.v_of = n, lhs_of, v_of
                self.tiles, self.pfx = tiles, pfx
                self.n_t = len(tiles)
                self.exs = {}
                self.pv = None
                self.exsum = None
                self.pvsb = None
                self.smsb = None

            def emit_qk(self, i):
                n, pfx = self.n, self.pfx
                s, w, W, m_ap, m_w = self.tiles[i]
                qk = psqk.tile([128, 2, TCH], F32, tag="qk",
                               name=f"qk{pfx}_{s}")
                for h in range(2):
                    nc.tensor.matmul(
                        qk[:, h, w:w + W], self.lhs_of(s),
                        qrot[:, h, bass.ds(n * TCH + w, W)],
                        start=True, stop=True)
                if m_ap is not None:
                    nc.vector.tensor_add(qk[:, :, w:w + m_w],
                                         qk[:, :, w:w + m_w], m_ap)
                ex = expp.tile([128, 2, TCH], BF16, tag="ex",
                               name=f"ex{pfx}_{s}")
                nc.scalar.activation(ex[:, :, w:w + W], qk[:, :, w:w + W],
                                     AF.Exp, scale=SCALE)
                self.exs[i] = ex

            def emit_pv(self, i):
                s, w, W, m_ap, m_w = self.tiles[i]
                ex = self.exs.pop(i)
                first = (i == 0)
                last = (i == self.n_t - 1)
                if first:
                    self.pv = pspv.tile([128, 2, TCH], F32, tag="pv",
                                        name=f"pv{self.pfx}")
                    self.exsum = exsp.tile([128, 2, TCH], F32R, tag="exsum",
                                           name=f"exs{self.pfx}")
                for h in range(2):
                    nc.tensor.matmul(self.pv[:, h, w:w + W], self.v_of(s),
                                     ex[:, h, w:w + W],
                                     start=first, stop=last)
                # running sum of exp tiles (DVE) -> one tiny PE colsum at
                # finish; halves the PE row count of the softmax epilogue
                if first:
                    nc.vector.tensor_copy(self.exsum[:], ex[:])
                else:
                    nc.vector.tensor_add(self.exsum[:, :, w:w + W],
                                         self.exsum[:, :, w:w + W],
                                         ex[:, :, w:w + W])
                if last:
                    self.finish()

            def finish(self):
                pfx = self.pfx
                # drain pv first: the next pass's first pv matmul blocks the
                # in-order PE until this frees the psum pair
                self.pvsb = drainp.tile([128, 2, TCH], BF16, tag="pvsb",
                                        name=f"pvsb{pfx}")
                nc.scalar.activation(self.pvsb[:], self.pv[:], AF.Copy)
                # denominators: colsum of exsum (h0 -> row 0, h1 -> row 1)
                smt = pssm.tile([2, TCH], F32, tag="sm", name=f"sm{pfx}")
                for h in range(2):
                    nc.tensor.matmul(smt[0:2, :], ones_r[:, 2 * h:2 * h + 2],
                                     self.exsum[:, h, :],
                                     start=(h == 0), stop=(h == 1))
                self.smsb = rcpp.tile([2, TCH], F32, tag="rcp2",
                                      name=f"smsb{pfx}")
                nc.scalar.activation(self.smsb[:], smt[:], AF.Copy)

        cstate = {}

        def combine_g(n, gp):
            # gate-weighted normalized global half: t1 = pv_g * gate/sum_g.
            # Runs right after the global pass finishes, hidden behind the
            # local pass, so the post-chunk serial tail is only combine_l.
            gsl_t = rcpp.tile([2, TCH], F32, tag="rcp2", name=f"gsl{n}")
            nc.sync.dma_start(out=gsl_t[:], in_=gate[2 * n:2 * n + 2, :])
            g1 = rcpp.tile([2, TCH], F32, tag="rcp2", name=f"g1{n}")
            nc.vector.tensor_scalar(g1[:], gsl_t[:], -1.0, 1.0,
                                    mybir.AluOpType.mult,
                                    mybir.AluOpType.add)
            rg = rcpp.tile([2, TCH], F32, tag="rcp2", name=f"rg{n}")
            nc.vector.reciprocal_approx_fast(rg[:], gp.smsb[:])
            ag = rcpp.tile([2, TCH], BF16, tag="rcpb", name=f"ag{n}")
            nc.vector.tensor_mul(ag[:], gsl_t[:], rg[:])
            # head-1 row to base partition 0 for partition_broadcast
            ag1 = rcpp.tile([1, TCH], BF16, tag="rcpb1", name=f"ag1{n}")
            nc.sync.dma_start(out=ag1[:], in_=ag[1:2, :])
            bg_t = bcp.tile([128, 2, TCH], BF16, tag="bcast", name=f"bg{n}")
            nc.gpsimd.partition_broadcast(bg_t[:, 0, :], ag[0:1, :])
            nc.gpsimd.partition_broadcast(bg_t[:, 1, :], ag1[:])
            t1 = combp.tile([128, 2, TCH], BF16, tag="comb", name=f"t1{n}")
            nc.vector.tensor_mul(t1[:], gp.pvsb[:], bg_t[:])
            cstate[n] = (g1, t1)

        def combine_l(n, lp):
            g1, t1 = cstate.pop(n)
            rl = rcpp.tile([2, TCH], F32, tag="rcp2", name=f"rl{n}")
            nc.vector.reciprocal_approx_fast(rl[:], lp.smsb[:])
            al = rcpp.tile([2, TCH], BF16, tag="rcpb", name=f"al{n}")
            nc.vector.tensor_mul(al[:], g1[:], rl[:])
            al1 = rcpp.tile([1, TCH], BF16, tag="rcpb1", name=f"al1{n}")
            nc.sync.dma_start(out=al1[:], in_=al[1:2, :])
            bl_t = bcp.tile([128, 2, TCH], BF16, tag="bcast", name=f"bl{n}")
            nc.gpsimd.partition_broadcast(bl_t[:, 0, :], al[0:1, :])
            nc.gpsimd.partition_broadcast(bl_t[:, 1, :], al1[:])
            t2 = combp.tile([128, 2, TCH], BF16, tag="comb", name=f"t2{n}")
            ao = aoutp.tile([128, 2, TCH], BF16, tag="aout", name=f"ao{n}")
            nc.vector.tensor_mul(t2[:], lp.pvsb[:], bl_t[:])
            nc.vector.tensor_add(ao[:], t1[:], t2[:])

            # ship finished 128-col blocks to a2a staging
            # token 1024+128c (hi) / 128c (lo) lives in chunk n at column
            # offset 128j; each unit covers 4 destination quarter-blocks
            buf = a2ai_hi if n >= 2 else a2ai_lo
            c0 = (n - 2) * 4 if n >= 2 else n * 4
            for h in range(2):
                for j in range(4):
                    nc.sync.dma_start(
                        out=buf[c0 + j, h * D:(h + 1) * D, :],
                        in_=ao[:, h, j * 128:(j + 1) * 128])

            if n == 2:
                # all-to-all #1: high-token halves (overlaps chunks 1,0)
                nc.gpsimd.collective_compute(
                    "AllToAll", mybir.AluOpType.bypass,
                    replica_groups=[list(range(NCORES))],
                    ins=[a2ai_hi[:].opt()], outs=[a2ao_hi[:].opt()])
                # stage its result immediately so o_proj-hi starts with no
                # gap the moment the attention stream drains
                nc.sync.dma_start(
                    out=afull_hi[:],
                    in_=a2ao_hi[:].rearrange("c p n -> (c p) n")
                        .rearrange("(k p) n -> p k n", p=128))

        units = []   # flat (pass, tile_idx) stream; post-pass hooks fire
        hooks = {}   # after the pv of a pass's last tile is emitted
        for n in reversed(range(NCH)):
            gtiles = []
            for s in range(0, 4 * n + 4):
                j = s - 4 * n
                if j < 0:
                    gtiles.append((s, 0, TCH, None, 0))
                else:
                    w = 128 * j
                    gtiles.append((s, w, TCH - w, trid_sb[:], 128))
            gp = Pass(n, lambda s: kgt_sb[:, s * 128:(s + 1) * 128],
                      lambda s: vg_sb[:, s, :], gtiles, f"g{n}")
            ltiles = []
            rlo = -1 if n > 0 else 0
            for r in range(rlo, 4):
                s = 4 * n + r
                if r == rlo:
                    m_ap = band1_sb[:] if r == -1 else band0_sb[:]
                    ltiles.append((s, 0, TCH, m_ap, TCH))
                elif r < 3:
                    w = 128 * r
                    ltiles.append((s, w, 256, band0_sb[:, :, 0:256], 256))
                else:
                    ltiles.append((s, 384, 128, band0_sb[:, :, 0:128], 128))
            lp = Pass(n, lambda s: krot[:, s * 128:(s + 1) * 128],
                      lambda s: vcur[:, s, :], ltiles, f"l{n}")
            units.extend((gp, i) for i in range(gp.n_t))
            hooks[len(units) - 1] = (combine_g, n, gp)
            units.extend((lp, i) for i in range(lp.n_t))
            hooks[len(units) - 1] = (combine_l, n, lp)

        DEPTH = 2
        for u, (p, i) in enumerate(units):
            # pv (and pass epilogue) before the new qk: its scalar-engine
            # drain then precedes the new tile's exp in the Act queue, so
            # the psum pair frees before the next pass needs it
            if u >= DEPTH:
                pp, pi = units[u - DEPTH]
                pp.emit_pv(pi)
                if (u - DEPTH) in hooks:
                    fn, hn, hp = hooks[u - DEPTH]
                    fn(hn, hp)
            p.emit_qk(i)
        for u in range(max(0, len(units) - DEPTH), len(units)):
            pp, pi = units[u]
            pp.emit_pv(pi)
            if u in hooks:
                fn, hn, hp = hooks[u]
                fn(hn, hp)

        pssm.release()
        pspv.release()
        psqk.release()
        expp.release()
        exsp.release()
        drainp.release()
        combp.release()
        bcp.release()
        rcpp.release()
        ropet.release()
        work.release()

        # ===== phase 3: all-to-all #2 (low halves) + split o_proj =====
        nc.gpsimd.collective_compute(
            "AllToAll", mybir.AluOpType.bypass,
            replica_groups=[list(range(NCORES))],
            ins=[a2ai_lo[:].opt()], outs=[a2ao_lo[:].opt()])

        pso = tc.alloc_tile_pool(name="pso", bufs=2, space="PSUM")

        nc.sync.dma_start(
            out=afull_lo[:],
            in_=a2ao_lo[:].rearrange("c p n -> (c p) n")
                .rearrange("(k p) n -> p k n", p=128))

        # ============ phase 4: o_proj for our token slice ============
        # hi half first: its matmuls run while all-to-all #2 is in flight.
        # OUT rows 0-127 = low half-slice, rows 128-255 = high half-slice
        for tt, afull in ((1, afull_hi), (0, afull_lo)):
            po = pso.tile([128, 4, TCH], F32, tag="po", name=f"po{tt}")
            for k in range(KT):
                for e in range(NCH):
                    nc.tensor.matmul(po[:, e, :],
                                     afull[:, k, :],
                                     wo_sb[:, k, e * TCH:(e + 1) * TCH],
                                     start=(k == 0), stop=(k == KT - 1))
            # per-e drains so each column block ships while the PE finishes
            # the remaining stop-matmuls / the other half's contraction
            for e in range(NCH):
                ot = osb.tile([128, TCH], BF16, tag="ot", name=f"ot{tt}_{e}")
                nc.scalar.activation(ot[:], po[:, e, :], AF.Copy)
                nc.sync.dma_start(
                    out=OUT[tt * 128:(tt + 1) * 128,
                            e * TCH:(e + 1) * TCH], in_=ot[:])
        pso.release()
        osb.release()
        wop.release()
        opool.release()
        aoutp.release()
        dram.release()
        const.release()

    nc.compile()
    return nc


def _host_prep(hidden_states, positions, k_global, v_global, w_qkv, w_o,
               w_gate, b_gate):
    """Layout-only host transforms + constant tables -> per-core in_maps."""
    f32 = np.float32
    bf = ml_dtypes.bfloat16
    hs = np.asarray(hidden_states, f32)
    pos = np.asarray(positions)
    kg = np.asarray(k_global, f32)
    vg = np.asarray(v_global, f32)
    wqkv = np.asarray(w_qkv, f32)
    wo = np.ascontiguousarray(np.asarray(w_o, f32).astype(bf))
    wg = np.asarray(w_gate, f32)
    bg = np.asarray(b_gate, f32)

    hst = np.ascontiguousarray(hs.T.astype(bf))

    half = D // 2
    inv_freq = (THETA ** (-np.arange(half, dtype=f32) / half)).astype(f32)
    ang = pos.astype(f32)[:, None] * inv_freq[None, :]
    cos_t = np.cos(ang).astype(f32).T       # [64, T]
    sin_t = np.sin(ang).astype(f32).T
    csf = np.ascontiguousarray(np.concatenate([cos_t, cos_t], axis=0).astype(bf))
    snf = np.ascontiguousarray(np.concatenate([-sin_t, sin_t], axis=0).astype(bf))

    p = np.arange(128, dtype=np.int64)[:, None]
    c128 = np.arange(128, dtype=np.int64)[None, :]
    c512 = np.arange(512, dtype=np.int64)[None, :]
    # TRI[p, c] = 0 iff c >= p  (diagonal 128-block of the causal mask)
    tri = np.where(c128 - p >= 0, 0.0, MASKV).astype(f32)
    # BAND0[p, c] = 0 iff 0 <= c - p <= WIN   (local band, d0 = 0)
    band0 = np.where((c512 - p >= 0) & (c512 - p <= WIN), 0.0, MASKV).astype(f32)
    # BAND1[p, c] = 0 iff 0 <= 128 + c - p <= WIN  (local band, d0 = 128)
    band1 = np.where((128 + c512 - p >= 0) & (128 + c512 - p <= WIN),
                     0.0, MASKV).astype(f32)
    trid = np.ascontiguousarray(np.concatenate([tri, tri], axis=1))
    band0d = np.ascontiguousarray(np.concatenate([band0, band0], axis=1))
    band1d = np.ascontiguousarray(np.concatenate([band1, band1], axis=1))

    on = np.ones((128, 1), f32)
    zo = np.zeros((128, 1), f32)
    ones2 = np.ascontiguousarray(np.concatenate([on, zo, zo, on], axis=1))
    idn = np.eye(128, dtype=f32).astype(bf)

    in_maps = []
    for c in range(NCORES):
        g = c // 2
        wq = wqkv[:, 2 * c * D:(2 * c + 2) * D]
        wk = wqkv[:, HQ * D + g * D:HQ * D + (g + 1) * D]
        wv = wqkv[:, (HQ + HK) * D + g * D:(HQ + HK) * D + (g + 1) * D]
        in_maps.append({
            "HST": hst,
            "WQKV": np.ascontiguousarray(
                np.concatenate([wq, wk, wv], axis=1).astype(bf)),
            "KGT": np.ascontiguousarray(kg[:, g * D:(g + 1) * D].T.astype(bf)),
            "VG": np.ascontiguousarray(vg[:, g * D:(g + 1) * D].astype(bf)),
            "WO": wo,
            "WG": np.ascontiguousarray(wg[:, 2 * c:2 * c + 2].astype(bf)),
            "BG": np.ascontiguousarray(bg[2 * c:2 * c + 2].reshape(1, 2)),
            "CSF": csf,
            "SNF": snf,
            "ONES2": ones2,
            "IDN": idn,
            "TRID": trid,
            "BAND0": band0d,
            "BAND1": band1d,
        })
    return in_maps


def kernel(**inputs):
    if "nc" not in _CACHE:
        _CACHE["nc"] = _build()
    nc = _CACHE["nc"]
    in_maps = _host_prep(**inputs)
    res = run_bass_kernel_spmd(nc, in_maps, core_ids=list(range(NCORES)))
    out = np.empty((T, HID), np.float32)
    for c in range(NCORES):
        o = np.asarray(res.results[c]["OUT"]).astype(np.float32)
        out[128 * c:128 * (c + 1)] = o[0:128]
        out[1024 + 128 * c:1024 + 128 * (c + 1)] = o[128:256]
    return out


# revision 40
# speedup vs baseline: 1.0549x; 1.0085x over previous
"""Trainium2 Bass kernel for LoopCoderAttention (sparse_attention).

Head-sharded tensor parallelism over 8 NeuronCores:
  core c owns query heads {2c, 2c+1} and KV head c//2.
All matmul operands are bf16 (PSUM accumulation stays fp32): enables FWL
fast weight loads, halves DMA/SBUF traffic, and keeps the PE stream dense
enough to hold the HAM clock-gate at 8/8 (2.4 GHz).

Attention is software-pipelined (depth-2 over s-tiles, both heads packed
into [128, 2, 512] PSUM tile pairs) with column-windowed diagonal/band
tiles so the PE only touches ~ the causally needed columns.

o_proj: a 2MB AllToAll reshards attention output from head-sharded to
token-sharded; the high-token half's o_proj overlaps the second AllToAll.
"""
import sys
sys.path.insert(0, '/opt/trn_rl_repo')
import numpy as np
import ml_dtypes
import concourse.bass as bass
import concourse.mybir as mybir
import concourse.tile as tile
from concourse import bacc
from concourse.bass_utils import run_bass_kernel_spmd

T = 2048
HID = 2048
HQ = 16
HK = 4
D = 128
WIN = 64
THETA = 10000.0
SCALE = D ** -0.5
NCORES = 8
TCH = 512                 # t-chunk (matmul free dim)
NCH = T // TCH            # 4 chunks
KT = HID // 128           # 16 k-tiles for 2048-deep contractions
ST = T // 128             # 16 s-tiles
TSL = T // NCORES         # 256-token output slice per core
MASKV = -1e9

F32 = mybir.dt.float32
F32R = mybir.dt.float32r
BF16 = mybir.dt.bfloat16
AF = mybir.ActivationFunctionType

_CACHE = {}


def _build():
    nc = bacc.Bacc("TRN2", target_bir_lowering=False, debug=False,
                   num_devices=NCORES)
    HST = nc.dram_tensor("HST", [HID, T], BF16, kind="ExternalInput").ap()
    WQKV = nc.dram_tensor("WQKV", [HID, 512], BF16, kind="ExternalInput").ap()
    KGT = nc.dram_tensor("KGT", [D, T], BF16, kind="ExternalInput").ap()
    VG = nc.dram_tensor("VG", [T, D], BF16, kind="ExternalInput").ap()
    WO = nc.dram_tensor("WO", [HID, HID], BF16, kind="ExternalInput").ap()
    WG = nc.dram_tensor("WG", [D, 2], BF16, kind="ExternalInput").ap()
    BG = nc.dram_tensor("BG", [1, 2], F32, kind="ExternalInput").ap()
    CSF = nc.dram_tensor("CSF", [128, T], BF16, kind="ExternalInput").ap()
    SNF = nc.dram_tensor("SNF", [128, T], BF16, kind="ExternalInput").ap()
    ONES2 = nc.dram_tensor("ONES2", [128, 4], F32, kind="ExternalInput").ap()
    IDN = nc.dram_tensor("IDN", [128, 128], BF16, kind="ExternalInput").ap()
    # head-doubled mask tables (identical halves for h0/h1)
    TRID = nc.dram_tensor("TRID", [128, 2 * 128], F32, kind="ExternalInput").ap()
    BAND0 = nc.dram_tensor("BAND0", [128, 2 * 512], F32, kind="ExternalInput").ap()
    BAND1 = nc.dram_tensor("BAND1", [128, 2 * 512], F32, kind="ExternalInput").ap()
    OUT = nc.dram_tensor("OUT", [TSL, HID], BF16, kind="ExternalOutput").ap()

    with tile.TileContext(nc) as tc:
        # pools are a strict stack: creation order is the reverse of the
        # release order at each phase boundary
        const = tc.alloc_tile_pool(name="const", bufs=1)
        dram = tc.alloc_tile_pool(name="dram", bufs=1, space="DRAM")
        aoutp = tc.alloc_tile_pool(name="aoutp", bufs=3)
        opool = tc.alloc_tile_pool(name="opool", bufs=1)
        wop = tc.alloc_tile_pool(name="wop", bufs=1)
        osb = tc.alloc_tile_pool(name="osb", bufs=2)
        work = tc.alloc_tile_pool(name="work", bufs=1)
        ropet = tc.alloc_tile_pool(name="ropet", bufs=2)
        rcpp = tc.alloc_tile_pool(name="rcpp", bufs=7)
        bcp = tc.alloc_tile_pool(name="bcp", bufs=2)
        combp = tc.alloc_tile_pool(name="combp", bufs=3)
        wqkvp = tc.alloc_tile_pool(name="wqkvp", bufs=1)
        chunkp = tc.alloc_tile_pool(name="chunkp", bufs=2)
        hsp = tc.alloc_tile_pool(name="hsp", bufs=8)
        ps1 = tc.alloc_tile_pool(name="ps1", bufs=7, space="PSUM")

        # ---- phase-1 constants first (critical path to first matmul) ----
        wqkv_sb = wqkvp.tile([128, KT, 512], BF16)
        wqkv_view = WQKV.rearrange("(k p) c -> p k c", p=128)
        hs_first = []
        n0 = NCH - 1
        for k in range(KT):
            # interleave weight/activation tiles so the k=0 matmul can
            # start after ~2 tile DMAs instead of after all 16 weight tiles
            nc.sync.dma_start(out=wqkv_sb[:, k, :], in_=wqkv_view[:, k, :])
            hs_t = hsp.tile([128, TCH], BF16, tag="hs_t", name=f"hsf{k}")
            nc.sync.dma_start(
                out=hs_t[:],
                in_=HST[k * 128:(k + 1) * 128, n0 * TCH:(n0 + 1) * TCH])
            hs_first.append(hs_t)
        csf_sb = wqkvp.tile([128, T], BF16)
        snf_sb = wqkvp.tile([128, T], BF16)
        idn_sb = wqkvp.tile([128, 128], BF16)
        wg_sb = const.tile([D, 2], BF16)
        nc.sync.dma_start(out=wg_sb[:], in_=WG)
        bg_sb = const.tile([1, 2], F32)
        nc.sync.dma_start(out=bg_sb[:], in_=BG)
        # attention-phase constants
        kgt_sb = const.tile([D, T], BF16)
        vg_sb = const.tile([128, ST, D], BF16)
        # [ones|zeros|zeros|ones]: col pair 0:2 sums h0 into psum row 0,
        # col pair 2:4 sums h1 into psum row 1 of the same bank (f32r to
        # match the f32r exsum accumulator it contracts against)
        ones_r = const.tile([128, 4], F32R)
        trid_sb = const.tile([128, 2, 128], F32)
        band0_sb = const.tile([128, 2, 512], F32)
        band1_sb = const.tile([128, 2, 512], F32)
        # o_proj weights (prefetched; consumed in phase 4)
        wo_sb = wop.tile([128, KT, HID], BF16)

        # ---- persistent work tiles (through attention) ----
        qrot = work.tile([128, 2, T], BF16)
        krot = work.tile([128, T], BF16)
        vcur = work.tile([128, ST, D], BF16)   # current v in [s, d] tiles
        gate = work.tile([8, TCH], F32)        # row 2n+h (DMA-staged access)

        a2ai_hi = dram.tile([NCORES, 2 * D, TSL // 2], BF16)
        a2ao_hi = dram.tile([NCORES, 2 * D, TSL // 2], BF16)
        a2ai_lo = dram.tile([NCORES, 2 * D, TSL // 2], BF16)
        a2ao_lo = dram.tile([NCORES, 2 * D, TSL // 2], BF16)
        a2ad_i = dram.tile([NCORES, 1, 4], BF16)
        a2ad_o = dram.tile([NCORES, 1, 4], BF16)

        def rope_chunk(dst_full, src, n):
            """dst_full[:, n*TCH:...] = neox-rope of chunk tile src [128, TCH].

            rot = src * [cos;cos] + rot90(src) * [-sin;sin], where rot90 swaps
            the two 64-partition halves (built with two SBUF->SBUF DMAs since
            DVE ops require matching base partitions).
            """
            sl = bass.ds(n * TCH, TCH)
            sr = ropet.tile([128, TCH], BF16, tag="ropesr", name=f"sr{n}")
            nc.sync.dma_start(out=sr[0:64, :], in_=src[64:128, :])
            nc.sync.dma_start(out=sr[64:128, :], in_=src[0:64, :])
            ta = ropet.tile([128, TCH], BF16, tag="ropetmp", name=f"ra{n}")
            tb = ropet.tile([128, TCH], BF16, tag="ropetmp", name=f"rb{n}")
            nc.vector.tensor_mul(ta[:], src[:], csf_sb[:, sl])
            nc.vector.tensor_mul(tb[:], sr[:], snf_sb[:, sl])
            nc.vector.tensor_add(dst_full[:, sl], ta[:], tb[:])

        # ================= phase 1: qkvT = wqkv^T @ hsT =================
        pending_small = []
        for n in reversed(range(NCH)):
            pss = [ps1.tile([128, TCH], F32, tag="ps1t", name=f"ps1_{n}_{m}")
                   for m in range(4)]
            hs_n = []
            for k in range(KT):
                if n == NCH - 1:
                    hs_t = hs_first[k]
                else:
                    hs_t = hsp.tile([128, TCH], BF16, tag="hs_t",
                                    name=f"hs_{n}_{k}")
                    nc.sync.dma_start(
                        out=hs_t[:],
                        in_=HST[k * 128:(k + 1) * 128,
                                n * TCH:(n + 1) * TCH])
                hs_n.append(hs_t)
            # m=3's psum bank is the previous chunk's most recently freed
            # slot; emit its first k-tiles after m=0..2's so the in-order PE
            # never blocks on the bank hand-off at the chunk boundary
            DEFER = 4 if n < NCH - 1 else 0
            order = [(k, m) for k in range(DEFER) for m in range(3)]
            order += [(k, 3) for k in range(DEFER)]
            order += [(k, m) for k in range(DEFER, KT) for m in range(4)]
            for k, m in order:
                nc.tensor.matmul(pss[m][:],
                                 wqkv_sb[:, k, m * 128:(m + 1) * 128],
                                 hs_n[k][:],
                                 start=(k == 0), stop=(k == KT - 1))
            if n == NCH - 1:
                # rope tables + identity: after the hot first-chunk DMAs,
                # before their first readers below
                nc.sync.dma_start(out=csf_sb[:], in_=CSF)
                nc.sync.dma_start(out=snf_sb[:], in_=SNF)
                nc.sync.dma_start(out=idn_sb[:], in_=IDN)
            if n == 1:
                # attention constants, needed right at attention start
                nc.sync.dma_start(out=kgt_sb[:], in_=KGT)
                nc.sync.dma_start(
                    out=vg_sb[:],
                    in_=VG.rearrange("(s p) d -> p s d", p=128))
                nc.sync.dma_start(out=ones_r[:], in_=ONES2.bitcast(F32R))
            if n == 0:
                nc.sync.dma_start(out=trid_sb[:],
                                  in_=TRID.rearrange("p (h c) -> p h c", h=2))
                nc.sync.dma_start(out=band0_sb[:],
                                  in_=BAND0.rearrange("p (h c) -> p h c", h=2))
                nc.sync.dma_start(out=band1_sb[:],
                                  in_=BAND1.rearrange("p (h c) -> p h c", h=2))
            # two-chunk deferral: a chunk's transposes/gates depend on its
            # own last qkv matmul + rope chain, so one chunk of emission
            # distance still stalls the in-order PE queue
            if len(pending_small) >= 2:
                pending_small.pop(0)()
            sl = bass.ds(n * TCH, TCH)
            q0c = chunkp.tile([128, TCH], BF16, tag="q0c")
            q1c = chunkp.tile([128, TCH], BF16, tag="q1c")
            kc = chunkp.tile([128, TCH], BF16, tag="kc")
            vc = chunkp.tile([128, TCH], BF16, tag="vc")
            nc.scalar.activation(q0c[:], pss[0][:], AF.Copy)
            nc.scalar.activation(q1c[:], pss[1][:], AF.Copy)
            nc.scalar.activation(kc[:], pss[2][:], AF.Copy)
            nc.vector.tensor_copy(vc[:], pss[3][:])

            rope_chunk(qrot[:, 0, :], q0c, n)
            rope_chunk(qrot[:, 1, :], q1c, n)
            rope_chunk(krot, kc, n)

            def small_ops(n=n, vc=vc, sl=sl):
                # v transposes + gates for chunk n: emitted one chunk later so
                # the PE stream never waits on the DVE rope/copy latency
                for j in range(4):
                    s = 4 * n + j
                    pt = ps1.tile([128, 128], BF16, tag="ps1g",
                                  name=f"pt{s}", bufs=1)
                    nc.tensor.transpose(pt[:], vc[:, j * 128:(j + 1) * 128],
                                        idn_sb[:])
                    nc.vector.tensor_copy(vcur[:, s, :], pt[:])
                for h in range(2):
                    r = 2 * n + h
                    gp = ps1.tile([1, TCH], F32, tag="ps1g",
                                  name=f"gp{r}", bufs=1)
                    nc.tensor.matmul(gp[:], wg_sb[:, h:h + 1], qrot[:, h, sl],
                                     start=True, stop=True)
                    gst = chunkp.tile([1, TCH], F32, tag="gst", name=f"gst{r}")
                    nc.scalar.activation(gst[:], gp[:], AF.Sigmoid,
                                         bias=bg_sb[0:1, h:h + 1])
                    nc.sync.dma_start(out=gate[r:r + 1, :], in_=gst[:])

            pending_small.append(small_ops)

        for f in pending_small:
            f()
        pending_small.clear()

        # prefetch o_proj weights now (after all phase-1 input DMAs queued)
        for k in range(KT):
            nc.sync.dma_start(out=wo_sb[:, k, :],
                              in_=WO[k * 128:(k + 1) * 128, :])

        ps1.release()
        hsp.release()
        chunkp.release()
        wqkvp.release()

        afull_hi = opool.tile([128, KT, TSL // 2], BF16)
        afull_lo = opool.tile([128, KT, TSL // 2], BF16)

        drainp = tc.alloc_tile_pool(name="drainp", bufs=3)
        exsp = tc.alloc_tile_pool(name="exsp", bufs=2)
        expp = tc.alloc_tile_pool(name="expp", bufs=4)
        psqk = tc.alloc_tile_pool(name="psqk", bufs=2, space="PSUM")
        pspv = tc.alloc_tile_pool(name="pspv", bufs=1, space="PSUM")
        pssm = tc.alloc_tile_pool(name="pssm", bufs=1, space="PSUM")

        # ============ phase 2: attention (global + local) ============
        # warm the collective path with a tiny all-to-all so the first real
        # one doesn't pay the ~11us cross-core trigger handshake
        nc.gpsimd.collective_compute(
            "AllToAll", mybir.AluOpType.bypass,
            replica_groups=[list(range(NCORES))],
            ins=[a2ad_i[:].opt()], outs=[a2ad_o[:].opt()])

        # chunks descend so the high-token half finishes first and its
        # all-to-all overlaps the low-token half's compute. All 8 passes are
        # software-pipelined into ONE flat PE stream (depth 2 across pass
        # boundaries) so the PE never idles long enough to re-throttle HAM.
        class Pass:
            def __init__(self, n, lhs_of, v_of, tiles, pfx):
                self.n, self.lhs_of, self.v_of = n, lhs_of, v_of
                self.tiles, self.pfx = tiles, pfx
                self.n_t = len(tiles)
                self.exs = {}
                self.pv = None
                self.exsum = None
                self.pvsb = None
                self.smsb = None

            def emit_qk(self, i):
                n, pfx = self.n, self.pfx
                s, w, W, m_ap, m_w = self.tiles[i]
                qk = psqk.tile([128, 2, TCH], F32, tag="qk",
                               name=f"qk{pfx}_{s}")
                for h in range(2):
                    nc.tensor.matmul(
                        qk[:, h, w:w + W], self.lhs_of(s),
                        qrot[:, h, bass.ds(n * TCH + w, W)],
                        start=True, stop=True)
                if m_ap is not None:
                    nc.vector.tensor_add(qk[:, :, w:w + m_w],
                                         qk[:, :, w:w + m_w], m_ap)
                ex = expp.tile([128, 2, TCH], BF16, tag="ex",
                               name=f"ex{pfx}_{s}")
                nc.scalar.activation(ex[:, :, w:w + W], qk[:, :, w:w + W],
                                     AF.Exp, scale=SCALE)
                self.exs[i] = ex

            def emit_pv(self, i):
                s, w, W, m_ap, m_w = self.tiles[i]
                ex = self.exs.pop(i)
                first = (i == 0)
                last = (i == self.n_t - 1)
                if first:
                    self.pv = pspv.tile([128, 2, TCH], F32, tag="pv",
                                        name=f"pv{self.pfx}")
                    self.exsum = exsp.tile([128, 2, TCH], F32R, tag="exsum",
                                           name=f"exs{self.pfx}")
                for h in range(2):
                    nc.tensor.matmul(self.pv[:, h, w:w + W], self.v_of(s),
                                     ex[:, h, w:w + W],
                                     start=first, stop=last)
                # running sum of exp tiles (DVE) -> one tiny PE colsum at
                # finish; halves the PE row count of the softmax epilogue
                if first:
                    nc.vector.tensor_copy(self.exsum[:], ex[:])
                else:
                    nc.vector.tensor_add(self.exsum[:, :, w:w + W],
                                         self.exsum[:, :, w:w + W],
                                         ex[:, :, w:w + W])
                if last:
                    self.finish()

            def finish(self):
                pfx = self.pfx
                # drain pv first: the next pass's first pv matmul blocks the
                # in-order PE until this frees the psum pair
                self.pvsb = drainp.tile([128, 2, TCH], BF16, tag="pvsb",
                                        name=f"pvsb{pfx}")
                nc.scalar.activation(self.pvsb[:], self.pv[:], AF.Copy)
                # denominators: colsum of exsum (h0 -> row 0, h1 -> row 1)
                smt = pssm.tile([2, TCH], F32, tag="sm", name=f"sm{pfx}")
                for h in range(2):
                    nc.tensor.matmul(smt[0:2, :], ones_r[:, 2 * h:2 * h + 2],
                                     self.exsum[:, h, :],
                                     start=(h == 0), stop=(h == 1))
                self.smsb = rcpp.tile([2, TCH], F32, tag="rcp2",
                                      name=f"smsb{pfx}")
                nc.scalar.activation(self.smsb[:], smt[:], AF.Copy)

        cstate = {}

        def combine_g(n, gp):
            # gate-weighted normalized global half: t1 = pv_g * gate/sum_g.
            # Runs right after the global pass finishes, hidden behind the
            # local pass, so the post-chunk serial tail is only combine_l.
            gsl_t = rcpp.tile([2, TCH], F32, tag="rcp2", name=f"gsl{n}")
            nc.sync.dma_start(out=gsl_t[:], in_=gate[2 * n:2 * n + 2, :])
            g1 = rcpp.tile([2, TCH], F32, tag="rcp2", name=f"g1{n}")
            nc.vector.tensor_scalar(g1[:], gsl_t[:], -1.0, 1.0,
                                    mybir.AluOpType.mult,
                                    mybir.AluOpType.add)
            rg = rcpp.tile([2, TCH], F32, tag="rcp2", name=f"rg{n}")
            nc.vector.reciprocal_approx_fast(rg[:], gp.smsb[:])
            ag = rcpp.tile([2, TCH], BF16, tag="rcpb", name=f"ag{n}")
            nc.vector.tensor_mul(ag[:], gsl_t[:], rg[:])
            # head-1 row to base partition 0 for partition_broadcast
            # head-1 row to base partition 0 (engines can't shift partitions)
            ag1 = rcpp.tile([1, TCH], BF16, tag="rcpb1", name=f"ag1{n}")
            nc.sync.dma_start(out=ag1[:], in_=ag[1:2, :])
            bg_t = bcp.tile([128, 2, TCH], BF16, tag="bcast", name=f"bg{n}")
            nc.gpsimd.partition_broadcast(bg_t[:, 0, :], ag[0:1, :])
            nc.gpsimd.partition_broadcast(bg_t[:, 1, :], ag1[:])
            t1 = combp.tile([128, 2, TCH], BF16, tag="comb", name=f"t1{n}")
            nc.vector.tensor_mul(t1[:], gp.pvsb[:], bg_t[:])
            cstate[n] = (g1, t1)

        def combine_l(n, lp):
            g1, t1 = cstate.pop(n)
            rl = rcpp.tile([2, TCH], F32, tag="rcp2", name=f"rl{n}")
            nc.vector.reciprocal_approx_fast(rl[:], lp.smsb[:])
            al = rcpp.tile([2, TCH], BF16, tag="rcpb", name=f"al{n}")
            nc.vector.tensor_mul(al[:], g1[:], rl[:])
            al1 = rcpp.tile([1, TCH], BF16, tag="rcpb1", name=f"al1{n}")
            nc.sync.dma_start(out=al1[:], in_=al[1:2, :])
            bl_t = bcp.tile([128, 2, TCH], BF16, tag="bcast", name=f"bl{n}")
            nc.gpsimd.partition_broadcast(bl_t[:, 0, :], al[0:1, :])
            nc.gpsimd.partition_broadcast(bl_t[:, 1, :], al1[:])
            t2 = combp.tile([128, 2, TCH], BF16, tag="comb", name=f"t2{n}")
            ao = aoutp.tile([128, 2, TCH], BF16, tag="aout", name=f"ao{n}")
            nc.vector.tensor_mul(t2[:], lp.pvsb[:], bl_t[:])
            nc.vector.tensor_add(ao[:], t1[:], t2[:])

            # ship finished 128-col blocks to a2a staging
            # token 1024+128c (hi) / 128c (lo) lives in chunk n at column
            # offset 128j; each unit covers 4 destination quarter-blocks
            buf = a2ai_hi if n >= 2 else a2ai_lo
            c0 = (n - 2) * 4 if n >= 2 else n * 4
            for h in range(2):
                for j in range(4):
                    nc.sync.dma_start(
                        out=buf[c0 + j, h * D:(h + 1) * D, :],
                        in_=ao[:, h, j * 128:(j + 1) * 128])

            if n == 2:
                # all-to-all #1: high-token halves (overlaps chunks 1,0)
                nc.gpsimd.collective_compute(
                    "AllToAll", mybir.AluOpType.bypass,
                    replica_groups=[list(range(NCORES))],
                    ins=[a2ai_hi[:].opt()], outs=[a2ao_hi[:].opt()])
                # stage its result immediately so o_proj-hi starts with no
                # gap the moment the attention stream drains
                nc.sync.dma_start(
                    out=afull_hi[:],
                    in_=a2ao_hi[:].rearrange("c p n -> (c p) n")
                        .rearrange("(k p) n -> p k n", p=128))

        units = []   # flat (pass, tile_idx) stream; post-pass hooks fire
        hooks = {}   # after the pv of a pass's last tile is emitted
        for n in reversed(range(NCH)):
            gtiles = []
            for s in range(0, 4 * n + 4):
                j = s - 4 * n
                if j < 0:
                    gtiles.append((s, 0, TCH, None, 0))
                else:
                    w = 128 * j
                    gtiles.append((s, w, TCH - w, trid_sb[:], 128))
            gp = Pass(n, lambda s: kgt_sb[:, s * 128:(s + 1) * 128],
                      lambda s: vg_sb[:, s, :], gtiles, f"g{n}")
            ltiles = []
            rlo = -1 if n > 0 else 0
            for r in range(rlo, 4):
                s = 4 * n + r
                if r == rlo:
                    m_ap = band1_sb[:] if r == -1 else band0_sb[:]
                    ltiles.append((s, 0, TCH, m_ap, TCH))
                elif r < 3:
                    w = 128 * r
                    ltiles.append((s, w, 256, band0_sb[:, :, 0:256], 256))
                else:
                    ltiles.append((s, 384, 128, band0_sb[:, :, 0:128], 128))
            lp = Pass(n, lambda s: krot[:, s * 128:(s + 1) * 128],
                      lambda s: vcur[:, s, :], ltiles, f"l{n}")
            units.extend((gp, i) for i in range(gp.n_t))
            hooks[len(units) - 1] = (combine_g, n, gp)
            units.extend((lp, i) for i in range(lp.n_t))
            hooks[len(units) - 1] = (combine_l, n, lp)

        DEPTH = 2
        for u, (p, i) in enumerate(units):
            # pv (and pass epilogue) before the new qk: its scalar-engine
            # drain then precedes the new tile's exp in the Act queue, so
            # the psum pair frees before the next pass needs it
            if u >= DEPTH:
                pp, pi = units[u - DEPTH]
                pp.emit_pv(pi)
                if (u - DEPTH) in hooks:
                    fn, hn, hp = hooks[u - DEPTH]
                    fn(hn, hp)
            p.emit_qk(i)
        for u in range(max(0, len(units) - DEPTH), len(units)):
            pp, pi = units[u]
            pp.emit_pv(pi)
            if u in hooks:
                fn, hn, hp = hooks[u]
                fn(hn, hp)

        pssm.release()
        pspv.release()
        psqk.release()
        expp.release()
        exsp.release()
        drainp.release()
        combp.release()
        bcp.release()
        rcpp.release()
        ropet.release()
        work.release()

        # ===== phase 3: all-to-all #2 (low halves) + split o_proj =====
        nc.gpsimd.collective_compute(
            "AllToAll", mybir.AluOpType.bypass,
            replica_groups=[list(range(NCORES))],
            ins=[a2ai_lo[:].opt()], outs=[a2ao_lo[:].opt()])

        pso = tc.alloc_tile_pool(name="pso", bufs=2, space="PSUM")

        nc.sync.dma_start(
            out=afull_lo[:],
            in_=a2ao_lo[:].rearrange("c p n -> (c p) n")
                .rearrange("(k p) n -> p k n", p=128))

        # ============ phase 4: o_proj for our token slice ============
        # hi half first: its matmuls run while all-to-all #2 is in flight.
        # OUT rows 0-127 = low half-slice, rows 128-255 = high half-slice
        for tt, afull in ((1, afull_hi), (0, afull_lo)):
            po = pso.tile([128, 4, TCH], F32, tag="po", name=f"po{tt}")
            for k in range(KT):
                for e in range(NCH):
                    nc.tensor.matmul(po[:, e, :],
                                     afull[:, k, :],
                                     wo_sb[:, k, e * TCH:(e + 1) * TCH],
                                     start=(k == 0), stop=(k == KT - 1))
            # per-e drains so each column block ships while the PE finishes
            # the remaining stop-matmuls / the other half's contraction
            for e in range(NCH):
                ot = osb.tile([128, TCH], BF16, tag="ot", name=f"ot{tt}_{e}")
                nc.vector.tensor_copy(ot[:], po[:, e, :])
                nc.sync.dma_start(
                    out=OUT[tt * 128:(tt + 1) * 128,
                            e * TCH:(e + 1) * TCH], in_=ot[:])
        pso.release()
        osb.release()
        wop.release()
        opool.release()
        aoutp.release()
        dram.release()
        const.release()

    nc.compile()
    return nc


def _host_prep(hidden_states, positions, k_global, v_global, w_qkv, w_o,
               w_gate, b_gate):
    """Layout-only host transforms + constant tables -> per-core in_maps."""
    f32 = np.float32
    bf = ml_dtypes.bfloat16
    hs = np.asarray(hidden_states, f32)
    pos = np.asarray(positions)
    kg = np.asarray(k_global, f32)
    vg = np.asarray(v_global, f32)
    wqkv = np.asarray(w_qkv, f32)
    wo = np.ascontiguousarray(np.asarray(w_o, f32).astype(bf))
    wg = np.asarray(w_gate, f32)
    bg = np.asarray(b_gate, f32)

    hst = np.ascontiguousarray(hs.T.astype(bf))

    half = D // 2
    inv_freq = (THETA ** (-np.arange(half, dtype=f32) / half)).astype(f32)
    ang = pos.astype(f32)[:, None] * inv_freq[None, :]
    cos_t = np.cos(ang).astype(f32).T       # [64, T]
    sin_t = np.sin(ang).astype(f32).T
    csf = np.ascontiguousarray(np.concatenate([cos_t, cos_t], axis=0).astype(bf))
    snf = np.ascontiguousarray(np.concatenate([-sin_t, sin_t], axis=0).astype(bf))

    p = np.arange(128, dtype=np.int64)[:, None]
    c128 = np.arange(128, dtype=np.int64)[None, :]
    c512 = np.arange(512, dtype=np.int64)[None, :]
    # TRI[p, c] = 0 iff c >= p  (diagonal 128-block of the causal mask)
    tri = np.where(c128 - p >= 0, 0.0, MASKV).astype(f32)
    # BAND0[p, c] = 0 iff 0 <= c - p <= WIN   (local band, d0 = 0)
    band0 = np.where((c512 - p >= 0) & (c512 - p <= WIN), 0.0, MASKV).astype(f32)
    # BAND1[p, c] = 0 iff 0 <= 128 + c - p <= WIN  (local band, d0 = 128)
    band1 = np.where((128 + c512 - p >= 0) & (128 + c512 - p <= WIN),
                     0.0, MASKV).astype(f32)
    trid = np.ascontiguousarray(np.concatenate([tri, tri], axis=1))
    band0d = np.ascontiguousarray(np.concatenate([band0, band0], axis=1))
    band1d = np.ascontiguousarray(np.concatenate([band1, band1], axis=1))

    on = np.ones((128, 1), f32)
    zo = np.zeros((128, 1), f32)
    ones2 = np.ascontiguousarray(np.concatenate([on, zo, zo, on], axis=1))
    idn = np.eye(128, dtype=f32).astype(bf)

    in_maps = []
    for c in range(NCORES):
        g = c // 2
        wq = wqkv[:, 2 * c * D:(2 * c + 2) * D]
        wk = wqkv[:, HQ * D + g * D:HQ * D + (g + 1) * D]
        wv = wqkv[:, (HQ + HK) * D + g * D:(HQ + HK) * D + (g + 1) * D]
        in_maps.append({
            "HST": hst,
            "WQKV": np.ascontiguousarray(
                np.concatenate([wq, wk, wv], axis=1).astype(bf)),
            "KGT": np.ascontiguousarray(kg[:, g * D:(g + 1) * D].T.astype(bf)),
            "VG": np.ascontiguousarray(vg[:, g * D:(g + 1) * D].astype(bf)),
            "WO": wo,
            "WG": np.ascontiguousarray(wg[:, 2 * c:2 * c + 2].astype(bf)),
            "BG": np.ascontiguousarray(bg[2 * c:2 * c + 2].reshape(1, 2)),
            "CSF": csf,
            "SNF": snf,
            "ONES2": ones2,
            "IDN": idn,
            "TRID": trid,
            "BAND0": band0d,
            "BAND1": band1d,
        })
    return in_maps


def kernel(**inputs):
    if "nc" not in _CACHE:
        _CACHE["nc"] = _build()
    nc = _CACHE["nc"]
    in_maps = _host_prep(**inputs)
    res = run_bass_kernel_spmd(nc, in_maps, core_ids=list(range(NCORES)))
    out = np.empty((T, HID), np.float32)
    for c in range(NCORES):
        o = np.asarray(res.results[c]["OUT"]).astype(np.float32)
        out[128 * c:128 * (c + 1)] = o[0:128]
        out[1024 + 128 * c:1024 + 128 * (c + 1)] = o[128:256]
    return out
